# revision 3
# baseline (speedup 1.0000x reference)
# Depthwise causal conv1d (B=8, T=4096, C=1024, K=4, dilation=1) on 8 TRN2
# NeuronCores.
#
# Math: y[b, t, c] = sum_{j=0..3} weight[c, 3-j] * x[b, t-j, c]   (x[t<0] = 0)
#
# Strategy (v2 — fp16 I/O, PE+DVE compute split):
#   - Shard batch: core b handles x[b] (one full (T, C) slice).
#   - Host transposes each shard to (C, T) and casts to fp16, so device DMA
#     traffic is halved vs fp32: 8.2MB in + 8.2MB out per core.  At the
#     360 GB/s per-core DMA roofline (shared by loads+stores) that's ~47us,
#     vs ~94us for the fp32 baseline.  fp16 keeps 11 sig bits: worst-case
#     abs err ~1e-2 against an output scale of ~3.2 (gate is 2e-2 rel).
#   - On-chip, per 128-channel block: one [128, T+4] fp16 tile (4-col zero
#     halo at the left edge for the causal pad).  The 8 512-col subtiles are
#     split between engines so compute hides under DMA:
#       * subtiles 0..4 -> TensorE: 4 accumulating matmuls with per-block
#         diagonal fp16 weights (PSUM does the tap sum); ACT copies
#         PSUM->SBUF with an inline fp32->fp16 cast.
#       * subtiles 5..7 -> DVE as one 1536-col slab: 4 tensor_scalar mults
#         (4x_2p mode: 4 elem/cycle/lane for 2-byte SBUF operands) + 3
#         tensor_tensor adds (2x_1p: 2 elem/cycle/lane).
#   - Loads ride the SP HWDGE ring, stores the ACT ring.
#   - Host casts fp16 results back to fp32 and re-transposes.

import numpy as np

B, T, C, K = 8, 4096, 1024, 4
N_CORES = 8
P = 128  # SBUF partitions
NSUB = 512  # PE subtile width (one fp32 PSUM bank)
HALO = 4  # leading zero columns (causal left pad), shipped from host
PE_SUB = 5  # subtiles 0..PE_SUB-1 on TensorE, the rest on DVE

_CACHE = {}


def _build_nc():
    import concourse.mybir as mybir
    import concourse.tile as tile
    from concourse import bacc
    from concourse.masks import make_identity

    f32 = mybir.dt.float32
    f16 = mybir.dt.float16
    add = mybir.AluOpType.add
    ncb = C // P  # channel blocks per core

    nc = bacc.Bacc(None)
    x = nc.declare_dram_parameter("x", [C, T + HALO], f16, isOutput=False)
    # w_sb[p, cb*K + jj] = weight[cb*128 + p, jj]
    w = nc.declare_dram_parameter("w", [P, ncb * K], f32, isOutput=False)
    y = nc.declare_dram_parameter("y", [C, T], f16, isOutput=True)

    s0 = PE_SUB * NSUB  # first DVE column
    L = T - s0  # DVE slab width

    with tile.TileContext(nc) as tc:
        with (
            tc.tile_pool(name="const", bufs=1) as cpool,
            tc.tile_pool(name="xin", bufs=3) as xpool,
            tc.tile_pool(name="yout", bufs=3) as ypool,
            tc.tile_pool(name="tmp", bufs=2) as tpool,
            tc.tile_pool(name="ps", bufs=8, space="PSUM") as pspool,
        ):
            w_sb = cpool.tile([P, ncb * K], f32)
            nc.sync.dma_start(out=w_sb[:, :], in_=w[:, :])
            ident = cpool.tile([P, P], f16)
            make_identity(nc, ident)

            half = T // 2  # store granularity: two [P, half] y tiles/block

            for cb in range(ncb):
                rows = slice(cb * P, (cb + 1) * P)
                xt = xpool.tile([P, T + HALO], f16)
                nc.sync.dma_start(out=xt[:, :], in_=x[rows, :])
                yt0 = ypool.tile([P, half], f16, tag="yt0")
                yt1 = ypool.tile([P, half], f16, tag="yt1")

                def wcol(j):
                    col = cb * K + (K - 1 - j)
                    return w_sb[:, col : col + 1]

                # wdiag[j] = diag(weight[cb*128 + p, K-1-j]) in fp16, built
                # on the otherwise-idle GpSimd engine (keeps DVE free and
                # lets block 0's matmuls start as soon as w_sb+ident land).
                wdiag = []
                for j in range(K):
                    wd = cpool.tile([P, P], f16, tag=f"wd_{cb}_{j}")
                    nc.gpsimd.tensor_scalar_mul(
                        out=wd[:, :], in0=ident[:, :], scalar1=wcol(j)
                    )
                    wdiag.append(wd)

                # --- TensorE subtiles ---
                for m in range(PE_SUB):
                    ps = pspool.tile([P, NSUB], f32)
                    for j in range(K):
                        off = HALO + NSUB * m - j
                        nc.tensor.matmul(
                            ps[:, :],
                            wdiag[j][:, :],
                            xt[:, off : off + NSUB],
                            start=(j == 0),
                            stop=(j == K - 1),
                        )
                    c0 = NSUB * m
                    if c0 < half:
                        dst = yt0[:, c0 : c0 + NSUB]
                    else:
                        dst = yt1[:, c0 - half : c0 - half + NSUB]
                    nc.scalar.copy(dst, ps[:, :])
                    if m == 3:
                        # first half of y is complete -> stream it out
                        nc.scalar.dma_start(out=y[rows, :half], in_=yt0[:, :])

                # --- DVE slab: y[:, s0:] = sum_j w_j * x[:, s0-j : s0-j+L] ---
                def xoff(j):
                    off = HALO + s0 - j
                    return xt[:, off : off + L]

                a = tpool.tile([P, L], f16, tag="a")
                bb = tpool.tile([P, L], f16, tag="b")
                cc = tpool.tile([P, L], f16, tag="c")
                dd = tpool.tile([P, L], f16, tag="d")
                nc.vector.tensor_scalar_mul(out=a[:, :], in0=xoff(0), scalar1=wcol(0))
                nc.vector.tensor_scalar_mul(out=bb[:, :], in0=xoff(1), scalar1=wcol(1))
                nc.vector.tensor_tensor(
                    out=a[:, :], in0=a[:, :], in1=bb[:, :], op=add
                )
                nc.vector.tensor_scalar_mul(out=cc[:, :], in0=xoff(2), scalar1=wcol(2))
                nc.vector.tensor_scalar_mul(out=dd[:, :], in0=xoff(3), scalar1=wcol(3))
                nc.vector.tensor_tensor(
                    out=cc[:, :], in0=cc[:, :], in1=dd[:, :], op=add
                )
                nc.vector.tensor_tensor(
                    out=yt1[:, s0 - half :], in0=a[:, :], in1=cc[:, :], op=add
                )

                nc.scalar.dma_start(out=y[rows, half:], in_=yt1[:, :])
    return nc


def _get_nc():
    if "nc" not in _CACHE:
        nc = _build_nc()
        nc.finalize()
        _CACHE["nc"] = nc
    return _CACHE["nc"]


def _pack_weight(weight):
    # w_sb[p, cb*K + jj] = weight[cb*P + p, jj]
    w = np.asarray(weight, dtype=np.float32)
    ncb = C // P
    return np.ascontiguousarray(
        w.reshape(ncb, P, K).transpose(1, 0, 2).reshape(P, ncb * K)
    )


def _prep_inputs(x, weight):
    x = np.asarray(x)
    w_sb = _pack_weight(weight)
    in_maps = []
    for b in range(N_CORES):
        xt = np.zeros((C, T + HALO), dtype=np.float16)
        xt[:, HALO:] = x[b].T
        in_maps.append({"x": xt, "w": w_sb})
    return in_maps


def _collect_output(res):
    y = np.empty((B, T, C), dtype=np.float32)
    for b in range(N_CORES):
        y[b] = res.results[b]["y"].T.astype(np.float32)
    return y


LAST_RESULT = None


def kernel(x, weight):
    global LAST_RESULT
    from concourse.bass_utils import run_bass_kernel_spmd

    in_maps = _prep_inputs(x, weight)
    nc = _get_nc()
    res = run_bass_kernel_spmd(nc, in_maps, list(range(N_CORES)))
    LAST_RESULT = res
    return _collect_output(res)


# revision 8
# speedup vs baseline: 1.6098x; 1.6098x over previous
# Depthwise causal conv1d (B=8, T=4096, C=1024, K=4, dilation=1) on 8 TRN2
# NeuronCores.
#
# Math: y[b, t, c] = sum_{j=0..3} weight[c, 3-j] * x[b, t-j, c]   (x[t<0] = 0)
#
# Strategy (v2 — fp16 I/O, PE+DVE compute split):
#   - Shard batch: core b handles x[b] (one full (T, C) slice).
#   - Host transposes each shard to (C, T) and casts to fp16, so device DMA
#     traffic is halved vs fp32: 8.2MB in + 8.2MB out per core.  At the
#     360 GB/s per-core DMA roofline (shared by loads+stores) that's ~47us,
#     vs ~94us for the fp32 baseline.  fp16 keeps 11 sig bits: worst-case
#     abs err ~1e-2 against an output scale of ~3.2 (gate is 2e-2 rel).
#   - On-chip, per 128-channel block: one [128, T+4] fp16 tile (4-col zero
#     halo at the left edge for the causal pad).  The 8 512-col subtiles are
#     split between engines so compute hides under DMA:
#       * subtiles 0..4 -> TensorE: 4 accumulating matmuls with per-block
#         diagonal fp16 weights (PSUM does the tap sum); ACT copies
#         PSUM->SBUF with an inline fp32->fp16 cast.
#       * subtiles 5..7 -> DVE as one 1536-col slab: 4 tensor_scalar mults
#         (4x_2p mode: 4 elem/cycle/lane for 2-byte SBUF operands) + 3
#         tensor_tensor adds (2x_1p: 2 elem/cycle/lane).
#   - Loads ride the SP HWDGE ring, stores the ACT ring.
#   - Host casts fp16 results back to fp32 and re-transposes.

import numpy as np

B, T, C, K = 8, 4096, 1024, 4
N_CORES = 8
P = 128  # SBUF partitions
NSUB = 512  # PE subtile width (one fp32 PSUM bank)
HALO = 4  # leading zero columns (causal left pad), shipped from host
PE_SUB = 5  # subtiles 0..PE_SUB-1 on TensorE, the rest on DVE

_CACHE = {}


def _build_nc():
    import concourse.mybir as mybir
    import concourse.tile as tile
    from concourse import bacc

    f32 = mybir.dt.float32
    f16 = mybir.dt.float16
    add = mybir.AluOpType.add
    ncb = C // P  # channel blocks per core

    nc = bacc.Bacc(None)
    x = nc.declare_dram_parameter("x", [C, T + HALO], f16, isOutput=False)
    # w_sb[p, cb*K + jj] = weight[cb*128 + p, jj]
    w = nc.declare_dram_parameter("w", [P, ncb * K], f32, isOutput=False)
    identity = nc.declare_dram_parameter("ident", [P, P], f16, isOutput=False)
    y = nc.declare_dram_parameter("y", [C, T], f16, isOutput=True)

    s0 = PE_SUB * NSUB  # first DVE column
    L = T - s0  # DVE slab width

    with tile.TileContext(nc) as tc:
        with (
            tc.tile_pool(name="const", bufs=1) as cpool,
            tc.tile_pool(name="xin", bufs=3) as xpool,
            tc.tile_pool(name="yout", bufs=3) as ypool,
            tc.tile_pool(name="tmp", bufs=2) as tpool,
            tc.tile_pool(name="ps", bufs=8, space="PSUM") as pspool,
        ):
            w_sb = cpool.tile([P, ncb * K], f32)
            nc.sync.dma_start(out=w_sb[:, :], in_=w[:, :])
            ident = cpool.tile([P, P], f16)
            nc.sync.dma_start(out=ident[:, :], in_=identity[:, :])

            def wcol_of(cb, j):
                col = cb * K + (K - 1 - j)
                return w_sb[:, col : col + 1]

            # wdiag[(cb, j)] = diag(weight[cb*128 + p, K-1-j]) in fp16.
            # Blocks 0-1 build on DVE up front (DVE is idle until the first
            # x tile lands anyway); later blocks build on ACT, emitted two
            # blocks ahead so PE never waits.  GpSimd is avoided entirely:
            # its tensor_scalar is ~14x slower than DVE's.
            wdiag = {}

            def build_wdiag(cb, eng):
                for j in range(K):
                    wd = cpool.tile([P, P], f16, tag=f"wd_{cb}_{j}", name="wd")
                    if eng == "dve":
                        nc.vector.tensor_scalar_mul(
                            out=wd[:, :], in0=ident[:, :], scalar1=wcol_of(cb, j)
                        )
                    else:
                        nc.scalar.mul(wd[:, :], ident[:, :], wcol_of(cb, j))
                    wdiag[(cb, j)] = wd

            build_wdiag(0, "dve")
            if ncb > 1:
                build_wdiag(1, "dve")

            half = T // 2  # store granularity: two [P, half] y tiles/block

            for cb in range(ncb):
                rows = slice(cb * P, (cb + 1) * P)
                xt = xpool.tile([P, T + HALO], f16)
                nc.sync.dma_start(out=xt[:, :], in_=x[rows, :])
                yt0 = ypool.tile([P, half], f16, tag="yt0")
                yt1 = ypool.tile([P, half], f16, tag="yt1")

                def wcol(j):
                    return wcol_of(cb, j)

                if cb + 2 < ncb:
                    build_wdiag(cb + 2, "act")

                # --- TensorE subtiles ---
                for m in range(PE_SUB):
                    ps = pspool.tile([P, NSUB], f32)
                    for j in range(K):
                        off = HALO + NSUB * m - j
                        nc.tensor.matmul(
                            ps[:, :],
                            wdiag[(cb, j)][:, :],
                            xt[:, off : off + NSUB],
                            start=(j == 0),
                            stop=(j == K - 1),
                        )
                    c0 = NSUB * m
                    if c0 < half:
                        dst = yt0[:, c0 : c0 + NSUB]
                    else:
                        dst = yt1[:, c0 - half : c0 - half + NSUB]
                    nc.scalar.copy(dst, ps[:, :])
                    if m == 3:
                        # first half of y is complete -> stream it out
                        nc.scalar.dma_start(out=y[rows, :half], in_=yt0[:, :])

                # --- DVE slab: y[:, s0:] = sum_j w_j * x[:, s0-j : s0-j+L] ---
                def xoff(j):
                    off = HALO + s0 - j
                    return xt[:, off : off + L]

                a = tpool.tile([P, L], f16, tag="a")
                bb = tpool.tile([P, L], f16, tag="b")
                cc = tpool.tile([P, L], f16, tag="c")
                dd = tpool.tile([P, L], f16, tag="d")
                nc.vector.tensor_scalar_mul(out=a[:, :], in0=xoff(0), scalar1=wcol(0))
                nc.vector.tensor_scalar_mul(out=bb[:, :], in0=xoff(1), scalar1=wcol(1))
                nc.vector.tensor_tensor(
                    out=a[:, :], in0=a[:, :], in1=bb[:, :], op=add
                )
                nc.vector.tensor_scalar_mul(out=cc[:, :], in0=xoff(2), scalar1=wcol(2))
                nc.vector.tensor_scalar_mul(out=dd[:, :], in0=xoff(3), scalar1=wcol(3))
                nc.vector.tensor_tensor(
                    out=cc[:, :], in0=cc[:, :], in1=dd[:, :], op=add
                )
                nc.vector.tensor_tensor(
                    out=yt1[:, s0 - half :], in0=a[:, :], in1=cc[:, :], op=add
                )

                nc.scalar.dma_start(out=y[rows, half:], in_=yt1[:, :])
    return nc


def _get_nc():
    if "nc" not in _CACHE:
        nc = _build_nc()
        nc.finalize()
        _CACHE["nc"] = nc
    return _CACHE["nc"]


def _pack_weight(weight):
    # w_sb[p, cb*K + jj] = weight[cb*P + p, jj]
    w = np.asarray(weight, dtype=np.float32)
    ncb = C // P
    return np.ascontiguousarray(
        w.reshape(ncb, P, K).transpose(1, 0, 2).reshape(P, ncb * K)
    )


def _prep_inputs(x, weight):
    x = np.asarray(x)
    w_sb = _pack_weight(weight)
    ident = np.eye(P, dtype=np.float16)
    in_maps = []
    for b in range(N_CORES):
        xt = np.zeros((C, T + HALO), dtype=np.float16)
        xt[:, HALO:] = x[b].T
        in_maps.append({"x": xt, "w": w_sb, "ident": ident})
    return in_maps


def _collect_output(res):
    y = np.empty((B, T, C), dtype=np.float32)
    for b in range(N_CORES):
        y[b] = res.results[b]["y"].T.astype(np.float32)
    return y


LAST_RESULT = None


def kernel(x, weight):
    global LAST_RESULT
    from concourse.bass_utils import run_bass_kernel_spmd

    in_maps = _prep_inputs(x, weight)
    nc = _get_nc()
    res = run_bass_kernel_spmd(nc, in_maps, list(range(N_CORES)))
    LAST_RESULT = res
    return _collect_output(res)


# revision 12
# speedup vs baseline: 1.7499x; 1.0870x over previous
# Depthwise causal conv1d (B=8, T=4096, C=1024, K=4, dilation=1) on 8 TRN2
# NeuronCores.
#
# Math: y[b, t, c] = sum_{j=0..3} weight[c, 3-j] * x[b, t-j, c]   (x[t<0] = 0)
#
# Strategy (v2 — fp16 I/O, PE+DVE compute split):
#   - Shard batch: core b handles x[b] (one full (T, C) slice).
#   - Host transposes each shard to (C, T) and casts to fp16, so device DMA
#     traffic is halved vs fp32: 8.2MB in + 8.2MB out per core.  At the
#     360 GB/s per-core DMA roofline (shared by loads+stores) that's ~47us,
#     vs ~94us for the fp32 baseline.  fp16 keeps 11 sig bits: worst-case
#     abs err ~1e-2 against an output scale of ~3.2 (gate is 2e-2 rel).
#   - On-chip, per 128-channel block: one [128, T+4] fp16 tile (4-col zero
#     halo at the left edge for the causal pad).  The 8 512-col subtiles are
#     split between engines so compute hides under DMA:
#       * subtiles 0..4 -> TensorE: 4 accumulating matmuls with per-block
#         diagonal fp16 weights (PSUM does the tap sum); ACT copies
#         PSUM->SBUF with an inline fp32->fp16 cast.
#       * subtiles 5..7 -> DVE as one 1536-col slab: 4 tensor_scalar mults
#         (4x_2p mode: 4 elem/cycle/lane for 2-byte SBUF operands) + 3
#         tensor_tensor adds (2x_1p: 2 elem/cycle/lane).
#   - Loads ride the SP HWDGE ring, stores the ACT ring.
#   - Host casts fp16 results back to fp32 and re-transposes.

import numpy as np

B, T, C, K = 8, 4096, 1024, 4
N_CORES = 8
P = 128  # SBUF partitions
NSUB = 512  # PE subtile width (one fp32 PSUM bank)
HALO = 4  # leading zero columns (causal left pad), shipped from host
PE_SUB = 5  # subtiles 0..PE_SUB-1 on TensorE, the rest on DVE

_CACHE = {}


def _build_nc():
    import concourse.mybir as mybir
    import concourse.tile as tile
    from concourse import bacc

    f32 = mybir.dt.float32
    f16 = mybir.dt.float16
    add = mybir.AluOpType.add
    ncb = C // P  # channel blocks per core

    nc = bacc.Bacc(None)
    x = nc.declare_dram_parameter("x", [C, T + HALO], f16, isOutput=False)
    # w_sb[p, cb*K + jj] = weight[cb*128 + p, jj]
    w = nc.declare_dram_parameter("w", [P, ncb * K], f32, isOutput=False)
    identity = nc.declare_dram_parameter("ident", [P, P], f16, isOutput=False)
    y = nc.declare_dram_parameter("y", [C, T], f16, isOutput=True)

    s0 = PE_SUB * NSUB  # first DVE column
    L = T - s0  # DVE slab width

    with tile.TileContext(nc) as tc:
        with (
            tc.tile_pool(name="const", bufs=1) as cpool,
            tc.tile_pool(name="xin", bufs=8) as xpool,
            tc.tile_pool(name="yout", bufs=8) as ypool,
            tc.tile_pool(name="tmp", bufs=2) as tpool,
            tc.tile_pool(name="ps", bufs=8, space="PSUM") as pspool,
        ):
            w_sb = cpool.tile([P, ncb * K], f32)
            nc.sync.dma_start(out=w_sb[:, :], in_=w[:, :])
            ident = cpool.tile([P, P], f16)
            nc.sync.dma_start(out=ident[:, :], in_=identity[:, :])

            def wcol_of(cb, j):
                col = cb * K + (K - 1 - j)
                return w_sb[:, col : col + 1]

            # wdiag[(cb, j)] = diag(weight[cb*128 + p, K-1-j]) in fp16.
            # Blocks 0-1 build on DVE up front (DVE is idle until the first
            # x tile lands anyway); later blocks build on ACT, emitted two
            # blocks ahead so PE never waits.  GpSimd is avoided entirely:
            # its tensor_scalar is ~14x slower than DVE's.
            wdiag = {}

            def build_wdiag(cb, eng):
                for j in range(K):
                    wd = cpool.tile([P, P], f16, tag=f"wd_{cb}_{j}", name="wd")
                    if eng == "dve":
                        nc.vector.tensor_scalar_mul(
                            out=wd[:, :], in0=ident[:, :], scalar1=wcol_of(cb, j)
                        )
                    else:
                        nc.scalar.mul(wd[:, :], ident[:, :], wcol_of(cb, j))
                    wdiag[(cb, j)] = wd

            build_wdiag(0, "dve")
            if ncb > 1:
                build_wdiag(1, "dve")

            half = T // 2  # store granularity: two [P, half] y tiles/block

            for cb in range(ncb):
                rows = slice(cb * P, (cb + 1) * P)
                # Two half-tiles per block (4-col overlap re-loaded) so the
                # first matmuls start after a 0.5MB load, every DMA moves a
                # uniform ~0.5MB grain, and deps are tracked per half.
                xta = xpool.tile([P, half + HALO], f16, tag="xta")
                xtb = xpool.tile([P, half + HALO], f16, tag="xtb")
                nc.sync.dma_start(out=xta[:, :], in_=x[rows, : half + HALO])
                nc.sync.dma_start(out=xtb[:, :], in_=x[rows, half : T + HALO])
                yt0 = ypool.tile([P, half], f16, tag="yt0")
                yt1 = ypool.tile([P, half], f16, tag="yt1")

                def wcol(j):
                    return wcol_of(cb, j)

                if cb + 2 < ncb:
                    build_wdiag(cb + 2, "act")

                # --- TensorE subtiles ---
                for m in range(PE_SUB):
                    ps = pspool.tile([P, NSUB], f32)
                    for j in range(K):
                        off = HALO + NSUB * m - j
                        if off + NSUB <= half + HALO:
                            rhs = xta[:, off : off + NSUB]
                        else:
                            rhs = xtb[:, off - half : off - half + NSUB]
                        nc.tensor.matmul(
                            ps[:, :],
                            wdiag[(cb, j)][:, :],
                            rhs,
                            start=(j == 0),
                            stop=(j == K - 1),
                        )
                    c0 = NSUB * m
                    if c0 < half:
                        dst = yt0[:, c0 : c0 + NSUB]
                    else:
                        dst = yt1[:, c0 - half : c0 - half + NSUB]
                    nc.scalar.copy(dst, ps[:, :])
                    if m == 3:
                        # first half of y is complete -> stream it out
                        nc.scalar.dma_start(out=y[rows, :half], in_=yt0[:, :])

                # --- DVE slab: y[:, s0:] = sum_j w_j * x[:, s0-j : s0-j+L] ---
                def xoff(j):
                    off = HALO + s0 - j - half
                    return xtb[:, off : off + L]

                a = tpool.tile([P, L], f16, tag="a")
                bb = tpool.tile([P, L], f16, tag="b")
                cc = tpool.tile([P, L], f16, tag="c")
                dd = tpool.tile([P, L], f16, tag="d")
                nc.vector.tensor_scalar_mul(out=a[:, :], in0=xoff(0), scalar1=wcol(0))
                nc.vector.tensor_scalar_mul(out=bb[:, :], in0=xoff(1), scalar1=wcol(1))
                nc.vector.tensor_tensor(
                    out=a[:, :], in0=a[:, :], in1=bb[:, :], op=add
                )
                nc.vector.tensor_scalar_mul(out=cc[:, :], in0=xoff(2), scalar1=wcol(2))
                nc.vector.tensor_scalar_mul(out=dd[:, :], in0=xoff(3), scalar1=wcol(3))
                nc.vector.tensor_tensor(
                    out=cc[:, :], in0=cc[:, :], in1=dd[:, :], op=add
                )
                nc.vector.tensor_tensor(
                    out=yt1[:, s0 - half :], in0=a[:, :], in1=cc[:, :], op=add
                )

                nc.scalar.dma_start(out=y[rows, half:], in_=yt1[:, :])
    return nc


def _get_nc():
    if "nc" not in _CACHE:
        nc = _build_nc()
        nc.finalize()
        _CACHE["nc"] = nc
    return _CACHE["nc"]


def _pack_weight(weight):
    # w_sb[p, cb*K + jj] = weight[cb*P + p, jj]
    w = np.asarray(weight, dtype=np.float32)
    ncb = C // P
    return np.ascontiguousarray(
        w.reshape(ncb, P, K).transpose(1, 0, 2).reshape(P, ncb * K)
    )


def _prep_inputs(x, weight):
    x = np.asarray(x)
    w_sb = _pack_weight(weight)
    ident = np.eye(P, dtype=np.float16)
    in_maps = []
    for b in range(N_CORES):
        xt = np.zeros((C, T + HALO), dtype=np.float16)
        xt[:, HALO:] = x[b].T
        in_maps.append({"x": xt, "w": w_sb, "ident": ident})
    return in_maps


def _collect_output(res):
    y = np.empty((B, T, C), dtype=np.float32)
    for b in range(N_CORES):
        y[b] = res.results[b]["y"].T.astype(np.float32)
    return y


LAST_RESULT = None


def kernel(x, weight):
    global LAST_RESULT
    from concourse.bass_utils import run_bass_kernel_spmd

    in_maps = _prep_inputs(x, weight)
    nc = _get_nc()
    res = run_bass_kernel_spmd(nc, in_maps, list(range(N_CORES)))
    LAST_RESULT = res
    return _collect_output(res)
